# revision 6
# baseline (speedup 1.0000x reference)
"""Trainium2 Bass kernel for ConstantTimeStrideAttention (CTSA).

Problem (hardcoded): B=2, S=4096, D=1536, H=12 heads, head dim d=128.
Each query s attends to 12 anchors: band offsets {+-1,+-2,+-3} (weight gw0),
{+-5,+-10} (weight gw1), and globals {0, S-1} (weight gw2 each), where
gw = softmax(group_scale).  softmax over the 12 anchor scores with additive
log-weights == multiplicative weights on exp(score).

Sharding: pure data parallel over (B=2) x (4 sequence chunks of 1024 rows)
-> 8 cores, no collectives.  Each core receives a 1056-row extended slice
of x (2 global rows + 14-left halo + 1024 own + 10-right halo + pad),
pre-transposed and cast to bf16 on the host.

On-core pipeline (everything bf16 on the PE, fp32 accumulation):
  1) qk^T projection: qkT[f, s] tiles via matmul(lhsT=Wqk^T, rhs=x^T).
     K^T written in a per-query-tile replicated "window" layout
     (8 slots x [160-wide window | 2 global cols]).  Q scaled by d^-0.5.
  2) v projection in natural layout: matmul(lhsT=x^T, rhs=Wv^T) -> V[s, f].
     (v bias is folded into a host-side constant: sum_j P = 1.)
  3) per (head h, query tile t of 128 rows):
     transposed scores S^T = matmul(lhsT=K^T window piece, rhs=Q^T tile),
     exp (ACT), multiply by banded weight mask (DVE), then
     A_nat = matmul(lhsT=P^T pieces, rhs=V pieces) and the softmax
     denominator via rhs=ones into the same PSUM tile.  Normalize with a
     per-partition reciprocal, transpose A via identity matmul -> A^T.
  4) out projection: Y^T = matmul(lhsT=Wo^T, rhs=A^T) -> fp32 out.
Host adds (b_v @ Wo^T + out_b) and stitches chunks together.
"""

import numpy as np
import ml_dtypes

import concourse.bass as bass
import concourse.mybir as mybir
import concourse.tile as tile
from concourse import bacc
from concourse.bass_utils import run_bass_kernel_spmd

BF16 = mybir.dt.bfloat16
F32 = mybir.dt.float32

B, S, D = 2, 4096, 1536
H, d = 12, 128
N_CORES = 8
CHUNK = 1024          # own rows per core
XROWS = 1056          # extended rows: 2 glob + 14 halo + 1024 + 10 halo + 6 pad
OWN0 = 16             # first own row inside x_ext
WIN = 160             # window width (keys) per query tile
SLOT = 162            # window + 2 global columns
NT = 8                # query tiles per core
ALPHA = float(d) ** -0.5

_prog_cache = {}


def _build_program():
    if "nc" in _prog_cache:
        return _prog_cache["nc"]

    nc = bacc.Bacc(
        "TRN2", target_bir_lowering=False, debug=False, num_devices=N_CORES)

    xT_d = nc.dram_tensor("xT", [D, XROWS], BF16, kind="ExternalInput")
    wqkvT_d = nc.dram_tensor("wqkvT", [D, 3 * D], BF16, kind="ExternalInput")
    woT_d = nc.dram_tensor("woT", [D, D], BF16, kind="ExternalInput")
    qkbias_d = nc.dram_tensor("qkbias", [128, 24], F32, kind="ExternalInput")
    wa_d = nc.dram_tensor("wa", [128, 3, 128], BF16, kind="ExternalInput")
    wbg_d = nc.dram_tensor("wbg", [34, 3, 128], BF16, kind="ExternalInput")
    ident_d = nc.dram_tensor("ident", [128, 128], BF16, kind="ExternalInput")
    ones_d = nc.dram_tensor("ones", [128, 1], BF16, kind="ExternalInput")
    yT_d = nc.dram_tensor("yT", [D, CHUNK], F32, kind="ExternalOutput")

    KO = D // 128  # 12 k-tiles along the contraction dim

    with tile.TileContext(nc) as tc:
        with (
            tc.tile_pool(name="persist", bufs=1) as persist,
            tc.tile_pool(name="wq", bufs=2) as wqp,
            tc.tile_pool(name="wv", bufs=2) as wvp,
            tc.tile_pool(name="wo", bufs=2) as wop,
            tc.tile_pool(name="work", bufs=3) as work,
            tc.tile_pool(name="yst", bufs=3) as yst,
            tc.tile_pool(name="proj_ps", bufs=3, space="PSUM") as proj_ps,
            tc.tile_pool(name="sc_ps", bufs=2, space="PSUM") as sc_ps,
            tc.tile_pool(name="ad_ps", bufs=2, space="PSUM") as ad_ps,
        ):
            # ---------- persistent SBUF tensors ----------
            xT = persist.tile([128, KO, XROWS], BF16)
            nc.sync.dma_start(xT[:], xT_d.rearrange("(ko p) s -> p ko s", p=128))

            qkbias = persist.tile([128, 24], F32)
            nc.gpsimd.dma_start(qkbias[:], qkbias_d[:])
            wa = persist.tile([128, 3, 128], BF16)
            nc.gpsimd.dma_start(wa[:], wa_d[:])
            wbg = persist.tile([34, 3, 128], BF16)
            nc.gpsimd.dma_start(wbg[:], wbg_d[:])
            ident = persist.tile([128, 128], BF16)
            nc.gpsimd.dma_start(ident[:], ident_d[:])
            ones = persist.tile([128, 1], BF16)
            nc.gpsimd.dma_start(ones[:], ones_d[:])

            QT = persist.tile([128, H, CHUNK], BF16)       # Q^T, s in [16,1040)
            KTw = persist.tile([128, H, NT * SLOT], BF16)  # K^T windows
            V = persist.tile([128, NT, D], BF16)           # V natural, s-tiles 0..7
            Vtail = persist.tile([34, NT, D], BF16)        # rows 0:32 tail, 32:34 glob
            Vglob = persist.tile([2, D], BF16)
            AT = persist.tile([128, H, CHUNK], BF16)       # attention out ^T

            wqkvT_v = wqkvT_d.rearrange("(ko p) f -> p ko f", p=128)
            woT_v = woT_d.rearrange("(ko p) f -> p ko f", p=128)

            # ---------- phase 1: q^T and k^T projections ----------
            for ft in range(24):
                w = wqp.tile([128, KO, 128], BF16, tag="wq")
                nc.sync.dma_start(w[:], wqkvT_v[:, :, ft * 128:(ft + 1) * 128])
                if ft < 12:
                    # q section: only own rows, s in [16, 1040)
                    for ncl in range(2):
                        ps = proj_ps.tile([128, 512], F32, tag="pps")
                        for kt in range(KO):
                            nc.tensor.matmul(
                                ps[:], w[:, kt, :],
                                xT[:, kt, OWN0 + ncl * 512: OWN0 + (ncl + 1) * 512],
                                start=(kt == 0), stop=(kt == KO - 1),
                            )
                        nc.scalar.activation(
                            QT[:, ft, ncl * 512:(ncl + 1) * 512], ps[:],
                            mybir.ActivationFunctionType.Identity,
                            bias=qkbias[:, ft:ft + 1], scale=ALPHA,
                        )
                else:
                    h = ft - 12
                    ktw = KTw[:, h, :].rearrange("p (t j) -> p t j", j=SLOT)
                    for ncl in range(3):
                        width = 512 if ncl < 2 else 32
                        ps = proj_ps.tile([128, 512], F32, tag="pps")
                        for kt in range(KO):
                            nc.tensor.matmul(
                                ps[:, :width], w[:, kt, :],
                                xT[:, kt, ncl * 512: ncl * 512 + width],
                                start=(kt == 0), stop=(kt == KO - 1),
                            )
                        psv = ps.rearrange("p (t j) -> p t j", j=128)
                        bias = qkbias[:, ft:ft + 1]
                        ident_fn = mybir.ActivationFunctionType.Identity
                        if ncl < 2:
                            # window heads: tiles 4c..4c+3, keys [128t, 128t+128)
                            t0 = 4 * ncl
                            nc.scalar.activation(
                                ktw[:, t0:t0 + 4, 0:128], psv[:, 0:4, :],
                                ident_fn, bias=bias,
                            )
                            # window tails: keys [128t+128, 128t+160)
                            if ncl == 0:
                                nc.scalar.activation(
                                    ktw[:, 0:3, 128:160], psv[:, 1:4, 0:32],
                                    ident_fn, bias=bias,
                                )
                                # global columns, replicated into all 8 slots
                                nc.scalar.activation(
                                    ktw[:, 0:NT, 160:162],
                                    ps[:, None, 0:2].to_broadcast([128, NT, 2]),
                                    ident_fn, bias=bias,
                                )
                            else:
                                nc.scalar.activation(
                                    ktw[:, 3:7, 128:160], psv[:, 0:4, 0:32],
                                    ident_fn, bias=bias,
                                )
                        else:
                            # last 32 cols (keys 1024..1056) = tail of tile 7
                            nc.scalar.activation(
                                ktw[:, 7:8, 128:160], ps[:, None, 0:32],
                                ident_fn, bias=bias,
                            )

            # ---------- phase 2: v projection (natural layout) ----------
            for fc in range(3):
                wv = wvp.tile([128, KO, 512], BF16, tag="wv")
                nc.sync.dma_start(
                    wv[:], wqkvT_v[:, :, 2 * D + fc * 512: 2 * D + (fc + 1) * 512])
                for st in range(9):
                    rows = 128 if st < 8 else 32
                    ps = proj_ps.tile([128, 512], F32, tag="pps")
                    for kt in range(KO):
                        nc.tensor.matmul(
                            ps[0:rows, :],
                            xT[:, kt, st * 128: st * 128 + rows], wv[:, kt, :],
                            start=(kt == 0), stop=(kt == KO - 1),
                        )
                    if st < 8:
                        nc.vector.tensor_copy(V[:, st, fc * 512:(fc + 1) * 512], ps[:])
                    if 1 <= st <= 8:
                        nc.vector.tensor_copy(
                            Vtail[0:32, st - 1, fc * 512:(fc + 1) * 512], ps[0:32, :])
                    if st == 0:
                        nc.vector.tensor_copy(
                            Vglob[:, fc * 512:(fc + 1) * 512], ps[0:2, :])
            # replicate global v rows into every tail slot (partition shift -> DMA)
            for t in range(NT):
                nc.sync.dma_start(Vtail[32:34, t, :], Vglob[:])

            # ---------- phase 3: attention ----------
            exp_fn = mybir.ActivationFunctionType.Exp
            for h in range(H):
                ktw = KTw[:, h, :].rearrange("p (t j) -> p t j", j=SLOT)
                for t in range(NT):
                    m = 0 if t == 0 else (2 if t == NT - 1 else 1)
                    qt = QT[:, h, t * 128:(t + 1) * 128]
                    sc = sc_ps.tile([128, 256], F32, tag="sc")
                    nc.tensor.matmul(sc[:, 0:128], ktw[:, t, 0:128], qt,
                                     start=True, stop=True)
                    nc.tensor.matmul(sc[0:34, 128:256], ktw[:, t, 128:162], qt,
                                     start=True, stop=True)
                    pa = work.tile([128, 128], BF16, tag="pa")
                    nc.scalar.activation(pa[:], sc[:, 0:128], exp_fn)
                    pbg = work.tile([34, 128], BF16, tag="pbg")
                    nc.scalar.activation(pbg[:], sc[0:34, 128:256], exp_fn)
                    pam = work.tile([128, 128], BF16, tag="pam")
                    nc.vector.tensor_mul(pam[:], pa[:], wa[:, m, :])
                    pbgm = work.tile([34, 128], BF16, tag="pbgm")
                    nc.vector.tensor_mul(pbgm[:], pbg[:], wbg[:, m, :])

                    ad = ad_ps.tile([128, 260], F32, tag="ad")
                    # A_nat = P^T.T @ V pieces ; den = P^T.T @ ones
                    nc.tensor.matmul(ad[:, 0:128], pam[:],
                                     V[:, t, h * 128:(h + 1) * 128],
                                     start=True, stop=False)
                    nc.tensor.matmul(ad[:, 0:128], pbgm[:],
                                     Vtail[:, t, h * 128:(h + 1) * 128],
                                     start=False, stop=True)
                    nc.tensor.matmul(ad[:, 128:129], pam[:], ones[:],
                                     start=True, stop=False)
                    nc.tensor.matmul(ad[:, 128:129], pbgm[:], ones[0:34, :],
                                     start=False, stop=True)

                    r = work.tile([128, 1], F32, tag="r")
                    nc.vector.reciprocal(r[:], ad[:, 128:129])
                    a_sb = work.tile([128, 128], BF16, tag="a_sb")
                    nc.vector.tensor_scalar_mul(a_sb[:], ad[:, 0:128], r[:])
                    # transpose: A^T = a_sb.T @ I
                    nc.tensor.matmul(ad[:, 132:260], a_sb[:], ident[:],
                                     start=True, stop=True)
                    nc.vector.tensor_copy(AT[:, h, t * 128:(t + 1) * 128],
                                          ad[:, 132:260])

            # ---------- phase 4: out projection ----------
            for ft in range(12):
                wo = wop.tile([128, KO, 128], BF16, tag="wo")
                nc.sync.dma_start(wo[:], woT_v[:, :, ft * 128:(ft + 1) * 128])
                for ncl in range(2):
                    ps = proj_ps.tile([128, 512], F32, tag="pps")
                    for kt in range(KO):
                        nc.tensor.matmul(
                            ps[:], wo[:, kt, :], AT[:, kt, ncl * 512:(ncl + 1) * 512],
                            start=(kt == 0), stop=(kt == KO - 1),
                        )
                    y = yst.tile([128, 512], F32, tag="y")
                    nc.scalar.copy(y[:], ps[:])
                    nc.sync.dma_start(
                        yT_d.rearrange("(fo p) s -> p fo s", p=128)
                        [:, ft, ncl * 512:(ncl + 1) * 512], y[:])

    nc.compile()
    _prog_cache["nc"] = nc
    return nc


def _host_prep(x, qkv_w, qkv_b, out_w, out_b, group_scale):
    """Build the per-core input maps (numpy only)."""
    bf16 = ml_dtypes.bfloat16
    g = np.asarray(group_scale, np.float64)
    e = np.exp(g - g.max())
    gw = (e / e.sum()).astype(np.float64)

    wqkvT = np.ascontiguousarray(qkv_w.astype(np.float32).T).astype(bf16)
    woT = np.ascontiguousarray(out_w.astype(np.float32).T).astype(bf16)

    qkbias = np.zeros((128, 24), np.float32)
    for ft in range(24):
        seg = qkv_b[ft * 128:(ft + 1) * 128].astype(np.float32)
        qkbias[:, ft] = seg * ALPHA if ft < 12 else seg

    ident = np.eye(128, dtype=bf16)
    ones = np.ones((128, 1), dtype=bf16)

    band = [(-1, 0), (1, 0), (-2, 0), (2, 0), (-3, 0), (3, 0),
            (-5, 1), (5, 1), (-10, 1), (10, 1)]

    in_maps = []
    for core in range(N_CORES):
        b, chunk = divmod(core, 4)
        c0 = chunk * CHUNK
        xe = np.zeros((XROWS, D), np.float32)
        xe[0] = x[b, 0]
        xe[1] = x[b, S - 1]
        if chunk > 0:
            xe[2:16] = x[b, c0 - 14:c0]
        xe[16:16 + CHUNK] = x[b, c0:c0 + CHUNK]
        if chunk < 3:
            xe[16 + CHUNK:26 + CHUNK] = x[b, c0 + CHUNK:c0 + CHUNK + 10]
        xT = np.ascontiguousarray(xe.T).astype(bf16)

        wa = np.zeros((128, 3, 128), np.float64)
        wbg = np.zeros((34, 3, 128), np.float64)
        for slot, t in ((0, 0), (1, 3), (2, NT - 1)):
            for p in range(128):
                s = c0 + 128 * t + p
                for off, grp in band:
                    a = min(max(s + off, 0), S - 1)
                    j = (a - c0 + 16) - 128 * t
                    wgt = gw[grp]
                    if j < 128:
                        wa[j, slot, p] += wgt
                    else:
                        wbg[j - 128, slot, p] += wgt
            wbg[32, slot, :] += gw[2]
            wbg[33, slot, :] += gw[2]

        in_maps.append({
            "xT": xT,
            "wqkvT": wqkvT,
            "woT": woT,
            "qkbias": qkbias,
            "wa": wa.astype(bf16),
            "wbg": wbg.astype(bf16),
            "ident": ident,
            "ones": ones,
        })

    y_const = (qkv_b[2 * D:3 * D].astype(np.float64) @
               out_w.astype(np.float64).T + out_b.astype(np.float64)
               ).astype(np.float32)
    return in_maps, y_const


def kernel(x, qkv_w, qkv_b, out_w, out_b, group_scale, _run_kwargs=None):
    x = np.asarray(x)
    in_maps, y_const = _host_prep(
        np.asarray(x, np.float32), np.asarray(qkv_w, np.float32),
        np.asarray(qkv_b, np.float32), np.asarray(out_w, np.float32),
        np.asarray(out_b, np.float32), np.asarray(group_scale, np.float32))
    nc = _build_program()
    kwargs = _run_kwargs or {}
    res = run_bass_kernel_spmd(nc, in_maps, core_ids=list(range(N_CORES)), **kwargs)
    out = np.empty((B, S, D), np.float32)
    for core in range(N_CORES):
        b, chunk = divmod(core, 4)
        r = res.results[core]
        yT = r["yT"] if isinstance(r, dict) else r
        out[b, chunk * CHUNK:(chunk + 1) * CHUNK] = np.asarray(yT, np.float32).T
    out += y_const
    if kwargs.get("trace"):
        kernel.last_exec_time_ns = res.exec_time_ns
    return out


if __name__ == "__main__":
    rng = np.random.default_rng(0)
    x = rng.standard_normal((B, S, D), dtype=np.float32)
    qkv_w = (rng.standard_normal((3 * D, D), dtype=np.float32) / np.sqrt(D))
    qkv_b = rng.standard_normal(3 * D, dtype=np.float32) * 0.01
    out_w = rng.standard_normal((D, D), dtype=np.float32) / np.sqrt(D)
    out_b = rng.standard_normal(D, dtype=np.float32) * 0.01
    gs = rng.standard_normal(3, dtype=np.float32)
    y = kernel(x=x, qkv_w=qkv_w, qkv_b=qkv_b, out_w=out_w, out_b=out_b,
               group_scale=gs)
    print("ok", y.shape, float(np.abs(y).mean()))


# revision 7
# speedup vs baseline: 1.2079x; 1.2079x over previous
"""Trainium2 Bass kernel for ConstantTimeStrideAttention (CTSA).

Problem (hardcoded): B=2, S=4096, D=1536, H=12 heads, head dim d=128.
Each query s attends to 12 anchors: band offsets {+-1,+-2,+-3} (weight gw0),
{+-5,+-10} (weight gw1), and globals {0, S-1} (weight gw2 each), where
gw = softmax(group_scale).  softmax over the 12 anchor scores with additive
log-weights == multiplicative weights on exp(score).

Sharding: pure data parallel over (B=2) x (4 sequence chunks of 1024 rows)
-> 8 cores, no collectives.  Each core receives a 1056-row extended slice
of x (2 global rows + 14-left halo + 1024 own + 10-right halo + pad),
pre-transposed and cast to bf16 on the host.

On-core pipeline (bf16 on the PE, fp32 accumulation):
  1) v projection in natural layout [key, feat], with a ones column per
     head so the AV matmul also produces the softmax denominator.
     (v bias is folded into a host-side constant: sum_j P == 1.)
  2) per head h: q^T/k^T projection tiles via matmul(lhsT=W^T, rhs=x^T);
     K^T is written in a per-query-tile replicated "window" layout
     (8 slots x [160-wide window | 2 global cols]); then attention for
     the previous head (keeps dense GEMM work interleaved with the
     sparse attention matmuls so the PE HAM clock stays at 2.4 GHz).
  3) attention per (h, query-tile t): transposed scores
     S^T = matmul(lhsT=K^T window pieces, rhs=Q^T tile) -> one exp (ACT)
     -> one banded-weight mask multiply (DVE) -> A_nat & denominator in
     one accumulation group (rhs = V pieces with ones column), normalize
     with per-partition reciprocal, transpose via identity matmul -> A^T.
  4) out projection: Y^T = matmul(lhsT=Wo^T, rhs=A^T) -> fp32 out.
Host adds (b_v @ Wo^T + out_b) and stitches chunks together.
"""

import numpy as np
import ml_dtypes

import concourse.bass as bass
import concourse.mybir as mybir
import concourse.tile as tile
from concourse import bacc
from concourse.bass_utils import run_bass_kernel_spmd

BF16 = mybir.dt.bfloat16
F32 = mybir.dt.float32

B, S, D = 2, 4096, 1536
H, d = 12, 128
N_CORES = 8
CHUNK = 1024          # own rows per core
XROWS = 1056          # extended rows: 2 glob + 14 halo + 1024 + 10 halo + 6 pad
OWN0 = 16             # first own row inside x_ext
WIN = 160             # window width (keys) per query tile
SLOT = 162            # window + 2 global columns
NT = 8                # query tiles per core
VS = 129              # per-(tile,head) V slot width: 128 features + ones col
ALPHA = float(d) ** -0.5

_prog_cache = {}


def _build_program():
    if "nc" in _prog_cache:
        return _prog_cache["nc"]

    nc = bacc.Bacc(
        "TRN2", target_bir_lowering=False, debug=False, num_devices=N_CORES)

    xT_d = nc.dram_tensor("xT", [D, XROWS], BF16, kind="ExternalInput")
    wqkvT_d = nc.dram_tensor("wqkvT", [D, 3 * D], BF16, kind="ExternalInput")
    woT_d = nc.dram_tensor("woT", [D, D], BF16, kind="ExternalInput")
    qkbias_d = nc.dram_tensor("qkbias", [128, 24], F32, kind="ExternalInput")
    wmask_d = nc.dram_tensor("wmask", [128, 3, 256], BF16, kind="ExternalInput")
    ident_d = nc.dram_tensor("ident", [128, 128], BF16, kind="ExternalInput")
    yT_d = nc.dram_tensor("yT", [D, CHUNK], F32, kind="ExternalOutput")

    KO = D // 128  # 12 k-tiles along the contraction dim
    ident_fn = mybir.ActivationFunctionType.Identity
    exp_fn = mybir.ActivationFunctionType.Exp

    with tile.TileContext(nc) as tc:
        with (
            tc.tile_pool(name="persist", bufs=1) as persist,
            tc.tile_pool(name="wq", bufs=2) as wqp,
            tc.tile_pool(name="wv", bufs=2) as wvp,
            tc.tile_pool(name="wo", bufs=2) as wop,
            tc.tile_pool(name="work", bufs=4) as work,
            tc.tile_pool(name="yst", bufs=3) as yst,
            tc.tile_pool(name="proj_ps", bufs=2, space="PSUM") as proj_ps,
            tc.tile_pool(name="sc_ps", bufs=3, space="PSUM") as sc_ps,
            tc.tile_pool(name="ad_ps", bufs=3, space="PSUM") as ad_ps,
        ):
            # ---------- persistent SBUF tensors ----------
            xT = persist.tile([128, KO, XROWS], BF16)
            nc.sync.dma_start(xT[:], xT_d.rearrange("(ko p) s -> p ko s", p=128))

            qkbias = persist.tile([128, 24], F32)
            nc.gpsimd.dma_start(qkbias[:], qkbias_d[:])
            wmask = persist.tile([128, 3, 256], BF16)
            nc.gpsimd.dma_start(wmask[:], wmask_d[:])
            ident = persist.tile([128, 128], BF16)
            nc.gpsimd.dma_start(ident[:], ident_d[:])

            QT = persist.tile([128, H, CHUNK], BF16)       # Q^T, s in [16,1040)
            KTw = persist.tile([128, H, NT * SLOT], BF16)  # K^T windows
            V = persist.tile([128, NT, H, VS], BF16)       # V natural + ones col
            Vtail = persist.tile([34, NT, H, VS], BF16)    # 32 tail rows + 2 glob
            Vglob = persist.tile([2, D], BF16)
            AT = persist.tile([128, H, CHUNK], BF16)       # attention out ^T

            nc.gpsimd.memset(V[:, :, :, 128:129], 1.0)
            nc.gpsimd.memset(Vtail[:, :, :, 128:129], 1.0)

            wqkvT_v = wqkvT_d.rearrange("(ko p) f -> p ko f", p=128)
            woT_v = woT_d.rearrange("(ko p) f -> p ko f", p=128)

            # ---------- phase 1: v projection (natural layout) ----------
            for fc in range(3):
                wv = wvp.tile([128, KO, 512], BF16, tag="wv")
                nc.sync.dma_start(
                    wv[:], wqkvT_v[:, :, 2 * D + fc * 512: 2 * D + (fc + 1) * 512])
                for st in range(9):
                    rows = 128 if st < 8 else 32
                    ps = proj_ps.tile([128, 512], F32, tag="pps")
                    for kt in range(KO):
                        nc.tensor.matmul(
                            ps[0:rows, :],
                            xT[:, kt, st * 128: st * 128 + rows], wv[:, kt, :],
                            start=(kt == 0), stop=(kt == KO - 1),
                        )
                    psv = ps.rearrange("p (h f) -> p h f", f=128)
                    if st < 8:
                        nc.vector.tensor_copy(
                            V[:, st, 4 * fc:4 * fc + 4, 0:128], psv[:])
                    if 1 <= st <= 8:
                        nc.vector.tensor_copy(
                            Vtail[0:32, st - 1, 4 * fc:4 * fc + 4, 0:128],
                            psv[0:32])
                    if st == 0:
                        nc.vector.tensor_copy(
                            Vglob[:, fc * 512:(fc + 1) * 512], ps[0:2, :])
            # replicate global v rows into every tail slot (partition shift -> DMA)
            vgv = Vglob.rearrange("p (h f) -> p h f", f=128)
            for t in range(NT):
                nc.sync.dma_start(Vtail[32:34, t, :, 0:128], vgv[:])

            # ---------- phase 2+3: per-head qk projection + attention ----------
            def qk_proj(h):
                # q section (f-tile h): own rows only, s in [16, 1040)
                w = wqp.tile([128, KO, 128], BF16, tag="wq")
                nc.sync.dma_start(w[:], wqkvT_v[:, :, h * 128:(h + 1) * 128])
                for ncl in range(2):
                    ps = proj_ps.tile([128, 512], F32, tag="pps")
                    for kt in range(KO):
                        nc.tensor.matmul(
                            ps[:], w[:, kt, :],
                            xT[:, kt, OWN0 + ncl * 512: OWN0 + (ncl + 1) * 512],
                            start=(kt == 0), stop=(kt == KO - 1),
                        )
                    # QT = (ps + bias) * alpha, on DVE
                    nc.vector.tensor_scalar(
                        QT[:, h, ncl * 512:(ncl + 1) * 512], ps[:],
                        qkbias[:, h:h + 1], ALPHA,
                        mybir.AluOpType.add, mybir.AluOpType.mult,
                    )
                # k section (f-tile 12+h): full extended rows, windowed layout
                ft = 12 + h
                w2 = wqp.tile([128, KO, 128], BF16, tag="wq")
                nc.sync.dma_start(w2[:], wqkvT_v[:, :, ft * 128:(ft + 1) * 128])
                ktw = KTw[:, h, :].rearrange("p (t j) -> p t j", j=SLOT)
                bias = qkbias[:, ft:ft + 1]
                for ncl in range(3):
                    width = 512 if ncl < 2 else 32
                    ps = proj_ps.tile([128, 512], F32, tag="pps")
                    for kt in range(KO):
                        nc.tensor.matmul(
                            ps[:, :width], w2[:, kt, :],
                            xT[:, kt, ncl * 512: ncl * 512 + width],
                            start=(kt == 0), stop=(kt == KO - 1),
                        )
                    psv = ps.rearrange("p (t j) -> p t j", j=128)
                    if ncl < 2:
                        t0 = 4 * ncl
                        nc.scalar.activation(
                            ktw[:, t0:t0 + 4, 0:128], psv[:, 0:4, :],
                            ident_fn, bias=bias)
                        if ncl == 0:
                            nc.scalar.activation(
                                ktw[:, 0:3, 128:160], psv[:, 1:4, 0:32],
                                ident_fn, bias=bias)
                            nc.scalar.activation(
                                ktw[:, 0:NT, 160:162],
                                ps[:, None, 0:2].to_broadcast([128, NT, 2]),
                                ident_fn, bias=bias)
                        else:
                            nc.scalar.activation(
                                ktw[:, 3:7, 128:160], psv[:, 0:4, 0:32],
                                ident_fn, bias=bias)
                    else:
                        nc.scalar.activation(
                            ktw[:, 7:8, 128:160], ps[:, None, 0:32],
                            ident_fn, bias=bias)

            def attention(h):
                ktw = KTw[:, h, :].rearrange("p (t j) -> p t j", j=SLOT)
                for t in range(NT):
                    m = 0 if t == 0 else (2 if t == NT - 1 else 1)
                    qt = QT[:, h, t * 128:(t + 1) * 128]
                    sc = sc_ps.tile([128, 256], F32, tag="sc")
                    nc.tensor.matmul(sc[:, 0:128], ktw[:, t, 0:128], qt,
                                     start=True, stop=True)
                    nc.tensor.matmul(sc[0:34, 128:256], ktw[:, t, 128:162], qt,
                                     start=True, stop=True)
                    pe = work.tile([128, 256], BF16, tag="pe")
                    nc.scalar.activation(pe[:], sc[:], exp_fn)
                    pm = work.tile([128, 256], BF16, tag="pm")
                    nc.vector.tensor_mul(pm[:], pe[:], wmask[:, m, :])

                    ad = ad_ps.tile([128, 260], F32, tag="ad")
                    nc.tensor.matmul(ad[:, 0:VS], pm[:, 0:128],
                                     V[:, t, h, :], start=True, stop=False)
                    nc.tensor.matmul(ad[:, 0:VS], pm[0:34, 128:256],
                                     Vtail[:, t, h, :], start=False, stop=True)

                    r = work.tile([128, 1], F32, tag="r")
                    nc.vector.reciprocal(r[:], ad[:, 128:129])
                    a_sb = work.tile([128, 128], BF16, tag="a_sb")
                    nc.vector.tensor_scalar_mul(a_sb[:], ad[:, 0:128], r[:])
                    # transpose: A^T = a_sb.T @ I
                    nc.tensor.matmul(ad[:, 132:260], a_sb[:], ident[:],
                                     start=True, stop=True)
                    nc.vector.tensor_copy(AT[:, h, t * 128:(t + 1) * 128],
                                          ad[:, 132:260])

            qk_proj(0)
            for h in range(1, H):
                qk_proj(h)
                attention(h - 1)
            attention(H - 1)

            # ---------- phase 4: out projection ----------
            for ft in range(12):
                wo = wop.tile([128, KO, 128], BF16, tag="wo")
                nc.sync.dma_start(wo[:], woT_v[:, :, ft * 128:(ft + 1) * 128])
                for ncl in range(2):
                    ps = proj_ps.tile([128, 512], F32, tag="pps")
                    for kt in range(KO):
                        nc.tensor.matmul(
                            ps[:], wo[:, kt, :], AT[:, kt, ncl * 512:(ncl + 1) * 512],
                            start=(kt == 0), stop=(kt == KO - 1),
                        )
                    y = yst.tile([128, 512], F32, tag="y")
                    nc.scalar.copy(y[:], ps[:])
                    nc.sync.dma_start(
                        yT_d.rearrange("(fo p) s -> p fo s", p=128)
                        [:, ft, ncl * 512:(ncl + 1) * 512], y[:])

    nc.compile()
    _prog_cache["nc"] = nc
    return nc


def _host_prep(x, qkv_w, qkv_b, out_w, out_b, group_scale):
    """Build the per-core input maps (numpy only)."""
    bf16 = ml_dtypes.bfloat16
    g = np.asarray(group_scale, np.float64)
    e = np.exp(g - g.max())
    gw = (e / e.sum()).astype(np.float64)

    wqkvT = np.ascontiguousarray(qkv_w.astype(np.float32).T).astype(bf16)
    woT = np.ascontiguousarray(out_w.astype(np.float32).T).astype(bf16)

    qkbias = np.zeros((128, 24), np.float32)
    for ft in range(24):
        qkbias[:, ft] = qkv_b[ft * 128:(ft + 1) * 128].astype(np.float32)

    ident = np.eye(128, dtype=bf16)

    band = [(-1, 0), (1, 0), (-2, 0), (2, 0), (-3, 0), (3, 0),
            (-5, 1), (5, 1), (-10, 1), (10, 1)]

    in_maps = []
    for core in range(N_CORES):
        b, chunk = divmod(core, 4)
        c0 = chunk * CHUNK
        xe = np.zeros((XROWS, D), np.float32)
        xe[0] = x[b, 0]
        xe[1] = x[b, S - 1]
        if chunk > 0:
            xe[2:16] = x[b, c0 - 14:c0]
        xe[16:16 + CHUNK] = x[b, c0:c0 + CHUNK]
        if chunk < 3:
            xe[16 + CHUNK:26 + CHUNK] = x[b, c0 + CHUNK:c0 + CHUNK + 10]
        xT = np.ascontiguousarray(xe.T).astype(bf16)

        # combined banded weight mask, [j, slot, 256]:
        #   cols 0:128 -> window piece a (keys 128t..128t+128)
        #   cols 128:256 rows 0:32 -> tail keys, rows 32:34 -> globals
        wm = np.zeros((128, 3, 256), np.float64)
        for slot, t in ((0, 0), (1, 3), (2, NT - 1)):
            for p in range(128):
                s = c0 + 128 * t + p
                for off, grp in band:
                    a = min(max(s + off, 0), S - 1)
                    j = (a - c0 + 16) - 128 * t
                    if j < 128:
                        wm[j, slot, p] += gw[grp]
                    else:
                        wm[j - 128, slot, 128 + p] += gw[grp]
            wm[32, slot, 128:256] += gw[2]
            wm[33, slot, 128:256] += gw[2]

        in_maps.append({
            "xT": xT,
            "wqkvT": wqkvT,
            "woT": woT,
            "qkbias": qkbias,
            "wmask": wm.astype(bf16),
            "ident": ident,
        })

    y_const = (qkv_b[2 * D:3 * D].astype(np.float64) @
               out_w.astype(np.float64).T + out_b.astype(np.float64)
               ).astype(np.float32)
    return in_maps, y_const


def kernel(x, qkv_w, qkv_b, out_w, out_b, group_scale, _run_kwargs=None):
    x = np.asarray(x)
    in_maps, y_const = _host_prep(
        np.asarray(x, np.float32), np.asarray(qkv_w, np.float32),
        np.asarray(qkv_b, np.float32), np.asarray(out_w, np.float32),
        np.asarray(out_b, np.float32), np.asarray(group_scale, np.float32))
    nc = _build_program()
    kwargs = _run_kwargs or {}
    res = run_bass_kernel_spmd(nc, in_maps, core_ids=list(range(N_CORES)), **kwargs)
    out = np.empty((B, S, D), np.float32)
    for core in range(N_CORES):
        b, chunk = divmod(core, 4)
        r = res.results[core]
        yT = r["yT"] if isinstance(r, dict) else r
        out[b, chunk * CHUNK:(chunk + 1) * CHUNK] = np.asarray(yT, np.float32).T
    out += y_const
    if kwargs.get("trace"):
        kernel.last_exec_time_ns = res.exec_time_ns
    return out


if __name__ == "__main__":
    rng = np.random.default_rng(0)
    x = rng.standard_normal((B, S, D), dtype=np.float32)
    qkv_w = (rng.standard_normal((3 * D, D), dtype=np.float32) / np.sqrt(D))
    qkv_b = rng.standard_normal(3 * D, dtype=np.float32) * 0.01
    out_w = rng.standard_normal((D, D), dtype=np.float32) / np.sqrt(D)
    out_b = rng.standard_normal(D, dtype=np.float32) * 0.01
    gs = rng.standard_normal(3, dtype=np.float32)
    y = kernel(x=x, qkv_w=qkv_w, qkv_b=qkv_b, out_w=out_w, out_b=out_b,
               group_scale=gs)
    print("ok", y.shape, float(np.abs(y).mean()))
